# revision 31
# baseline (speedup 1.0000x reference)
# Neural-collapse regularizer (tr_SW / tr_SB) on 8 TRN2 NeuronCores.
#
# Math: with per-class sums S_c = sum_{i: l_i=c} x_i, counts n_c,
# ssq = sum_i ||x_i||^2:
#   tr_SW = ssq - sum_c ||S_c||^2 / n_c
#   tr_SB = sum_c ||S_c/n_c - g||^2,  g = (sum_c S_c) / N
# The device computes the segment sums [C, D] and ssq; everything else
# is tiny O(C*D) host math.
#
# Sharding: class-parallel. Core k owns classes [128k, 128(k+1)); the
# host routes each row to the core that owns its label.
#
# Layout: rows are packed in chunks of GRP=8 rows of one class, one
# chunk per (group, partition) slot; features ship as fp8e4 (halves
# HBM traffic vs bf16; ~2e-3 error on the final ratio vs the 2e-2
# gate). Each group's 4352B partition line = 8 x-tiles (8*512) + the
# slot's one-hot duplicated twice (2*128), so the one-hot costs no
# compute and no extra DMA round. Tiles j and j+4 form a "pair"
# [128, 2, 512] feeding fp8 DoubleRow matmuls (2 k-tiles per pass).
#
# Per group (8 tiles = 4 pairs):
#   PE  : pairs {0,1} -> Gram-diagonal ssq (accumulate X_b^T X_b for
#         the four 128-col blocks into ONE [128,128] psum across all
#         groups; its diagonal sums to those pairs' ssq), plus 4
#         DoubleRow class-sum matmuls sharing the shipped one-hot.
#   DVE : pair 2 -> scalar_tensor_tensor square with fused accum_out
#         (per-group per-partition partials; wide output is a dump).
#   ACT : pair 3 -> Square activation with fused accum_out.
# ssq = gram diag + DVE accums + ACT accums, reduced in a short tail.

import contextlib
import ctypes
import os
import sys
import types

import numpy as np
import ml_dtypes

import concourse.bass as bass
import concourse.bacc as bacc
import concourse.mybir as mybir
import concourse.bass_utils as _bass_utils
from concourse.bass_utils import run_bass_kernel_spmd

# Compile with walrus's ldweights optimization (elides redundant weight
# reloads, e.g. the 4 class-sum matmuls per group sharing one one-hot).
# The concourse default pins it off; flip the flag on the compiler cmdline.
if not getattr(_bass_utils, "_ldw_opt_patched", False):
    _orig_run_command = _bass_utils.run_command

    def _run_command_ldw_opt(cmd, **kw):
        cmd = ["--enable-ldw-opt=true" if c == "--enable-ldw-opt=false" else c
               for c in cmd]
        return _orig_run_command(cmd, **kw)

    _bass_utils.run_command = _run_command_ldw_opt
    _bass_utils._ldw_opt_patched = True


def _ensure_ntff_hook():
    """Provide antenv.axon_hooks + an NTFF profile hook when the image's
    antenv package lacks it (needed only for trace=True timing runs)."""
    try:
        from antenv.axon_hooks import get_axon_ntff_profile_hook  # noqa: F401
        return
    except ImportError:
        pass
    mod = types.ModuleType("antenv.axon_hooks")
    state = {"hook": None}
    mod.set_axon_ntff_profile_hook = lambda h: state.__setitem__("hook", h)
    mod.get_axon_ntff_profile_hook = lambda: state["hook"]
    sys.modules["antenv.axon_hooks"] = mod

    so_path = "/opt/axon/libaxon_pjrt.so"
    if not os.path.exists(so_path):
        return
    lib = ctypes.CDLL(so_path)
    if not hasattr(lib, "axon_start_nrt_profile"):
        return
    lib.axon_start_nrt_profile.argtypes = [
        ctypes.POINTER(ctypes.c_int64), ctypes.c_size_t]
    lib.axon_start_nrt_profile.restype = ctypes.c_int64
    lib.axon_stop_nrt_profile.argtypes = [ctypes.c_char_p]
    lib.axon_stop_nrt_profile.restype = ctypes.c_int64

    @contextlib.contextmanager
    def _hook(output_dir, device_ids):
        import jax
        jax.devices()
        if device_ids:
            ids = (ctypes.c_int64 * len(device_ids))(*device_ids)
            rc = lib.axon_start_nrt_profile(ids, len(device_ids))
        else:
            rc = lib.axon_start_nrt_profile(None, 0)
        if rc != 0:
            raise RuntimeError(f"axon_start_nrt_profile rc={rc}")
        try:
            yield
        finally:
            n = lib.axon_stop_nrt_profile(str(output_dir).encode())
            print(f"profile: {n} file(s) written to {output_dir}",
                  file=sys.stderr)

    mod.set_axon_ntff_profile_hook(_hook)


CORES = 8
P = 128              # partitions = classes per core
D = 512              # feature dim (asserted against input)
GRP = 8              # row-tiles per group = rows per chunk
HALF = D // 2
LINE = GRP * D           # 4096: 8 x-tiles per group line
BF16 = mybir.dt.bfloat16
F32 = mybir.dt.float32
FP8 = mybir.dt.float8e4
NP_BF16 = ml_dtypes.bfloat16
NP_FP8 = ml_dtypes.float8_e4m3

# out columns: 512 class sums, 512=gram diag, 513=ACT accums, 514=DVE
# accums
OUTW = D + 3

XB = int(os.environ.get("K_XB", "9"))         # x buffers (2 groups each)
NWARM = int(os.environ.get("K_NWARM", "24"))  # PE clock-ramp dummies


def _host_shard(features: np.ndarray, labels: np.ndarray):
    """Chunked class-sorted layout.

    Returns (in_maps, G). in_maps[k]:
      feat:  [G, 128, LINE] fp8e4 -- slot (g, p): GRP rows of one class
             at j*D offsets, then its one-hot (0..127) duplicated 2x.
      id128: [128, 128] f32 -- identity matrix (gram diag extraction)
    """
    N, d = features.shape
    assert d == D, f"expected D={D}, got {d}"
    CPAD = CORES * P

    order = np.argsort(labels, kind="stable")
    sl = labels[order]
    class_start = np.searchsorted(sl, np.arange(CPAD + 1))  # [1025]
    counts = np.diff(class_start)                            # [1024]
    chunks_per_class = -(-counts // GRP)                     # ceil
    core_chunks = chunks_per_class.reshape(CORES, P)
    G = int(-(-core_chunks.sum(axis=1).max() // P))

    f8 = features.astype(NP_FP8)
    eye = np.eye(P, dtype=NP_FP8)
    eye2 = np.concatenate([eye, eye], axis=1)                # [128, 256]
    id128 = np.eye(P, dtype=np.float32)

    in_maps = []
    for k in range(CORES):
        nch = core_chunks[k]                    # chunks per rebased class
        total = int(nch.sum())
        assert total <= G * P
        chunk_cls = np.repeat(np.arange(P), nch)             # [total]
        # padded row grid: [G*P, GRP] of global row indices, -1 = empty
        grid = np.full((G * P, GRP), -1, dtype=np.int64)
        cls_pad_start = np.concatenate(([0], np.cumsum(nch * GRP)))  # [129]
        cnts = counts[k * P:(k + 1) * P]
        lo = class_start[k * P]
        n_k = int(cnts.sum())
        rows_k = order[lo:lo + n_k]
        within = np.arange(n_k) - np.repeat(class_start[k * P:(k + 1) * P] - lo,
                                            cnts)
        pos = np.repeat(cls_pad_start[:-1], cnts) + within
        grid.reshape(-1)[pos] = rows_k

        safe = np.maximum(grid, 0)
        fr = f8[safe.reshape(-1)]               # [G*P*GRP, D]
        fr[grid.reshape(-1) < 0] = 0

        slot_cls = np.zeros((G * P,), dtype=np.int64)
        slot_cls[:total] = chunk_cls

        feat = fr.reshape(G, P, LINE)
        ohall = np.ascontiguousarray(
            eye2[slot_cls.reshape(G, P)].transpose(1, 0, 2)
        ).reshape(P, G * 2 * P)
        # pack group pairs into one 2*LINE partition line (fewer, larger
        # DMA batches); odd G -> pad with a never-transferred dummy half
        G2 = (G + 1) // 2
        feat2 = np.zeros((G2, P, 2, LINE), dtype=NP_FP8)
        for g in range(G):
            feat2[g // 2, :, g % 2, :] = feat[g]
        feat2 = np.ascontiguousarray(feat2.reshape(G2, P, 2 * LINE))

        in_maps.append({"feat": feat2, "oh": ohall, "id128": id128})
    return in_maps, G


def _build_raw(G: int):
    nc = bacc.Bacc("TRN2", target_bir_lowering=False, debug=False,
                   enable_asserts=False)
    G2 = (G + 1) // 2
    feat_h = nc.dram_tensor("feat", [G2, P, 2 * LINE], FP8,
                            kind="ExternalInput")
    oh_h = nc.dram_tensor("oh", [P, G * 2 * P], FP8, kind="ExternalInput")
    id_h = nc.dram_tensor("id128", [P, P], F32, kind="ExternalInput")
    out_h = nc.dram_tensor("out", [P, OUTW], F32, kind="ExternalOutput")

    x_sb = nc.alloc_sbuf_tensor("x_sb", [P, XB, 2, GRP, D], FP8)
    oh_sb = nc.alloc_sbuf_tensor("oh_sb", [P, G, 2, P], FP8)
    dumpd_sb = nc.alloc_sbuf_tensor("dumpd_sb", [P, 2, 2, 2, D], BF16)
    dumpa_sb = nc.alloc_sbuf_tensor("dumpa_sb", [P, 2, 6 * D], BF16)
    acca_sb = nc.alloc_sbuf_tensor("acca_sb", [P, G2], F32)
    accd_sb = nc.alloc_sbuf_tensor("accd_sb", [P, G2], F32)
    gd_sb = nc.alloc_sbuf_tensor("gd_sb", [P, P], F32)
    gd2_sb = nc.alloc_sbuf_tensor("gd2_sb", [P, P], F32)
    id_sb = nc.alloc_sbuf_tensor("id_sb", [P, P], F32)
    warm_sb = nc.alloc_sbuf_tensor("warm_sb", [P, 64], BF16)
    out_sb = nc.alloc_sbuf_tensor("out_sb", [P, OUTW], F32)
    psum_cls = nc.alloc_psum_tensor("psum_cls", [P, D], F32)
    psum_gram = nc.alloc_psum_tensor("psum_gram", [P, P], F32)
    psum_warm = nc.alloc_psum_tensor("psum_warm", [P, 64], F32)

    DR = mybir.MatmulPerfMode.DoubleRow

    import contextlib as _ctx
    with (
        _ctx.ExitStack() as _sems,
        nc.semaphore("sem_sqd") as sem_sqd,
        nc.semaphore("sem_sqa") as sem_sqa,
        nc.semaphore("sem_pe") as sem_pe,
        nc.semaphore("sem_cp") as sem_cp,
        nc.semaphore("sem_out") as sem_out,
        nc.semaphore("sem_warm") as sem_warm,
        nc.semaphore("sem_gd") as sem_gd,
        nc.semaphore("sem_oh") as sem_oh,
        nc.semaphore("sem_g1") as sem_g1,
        nc.semaphore("sem_id") as sem_id,
        nc.Block() as block,
    ):
        sem_xs = [_sems.enter_context(nc.semaphore(f"sem_x{b}"))
                  for b in range(XB)]

        def wait_x(eng, g):
            # supergroup 0 is split into two half-line DMAs (own sems)
            if g == 1:
                eng.wait_ge(sem_g1, 16)
                return
            sg = g // 2
            eng.wait_ge(sem_xs[sg % XB], 16 * (sg // XB + 1))

        def xline(g):
            return x_sb.ap()[:, (g // 2) % XB, g % 2]

        # pair t of group g: tiles (t, t+4) -> AP [128, 2, 512]
        def xpair(g, t):
            return xline(g)[:, t:t + 5:4, :]

        def xpair_blk(g, t, b):
            return xline(g)[:, t:t + 5:4, b * P:(b + 1) * P]

        # DVE/ACT consume a whole supergroup's pair in one op:
        # [128, groups(1|2), 2, 512]
        def xsg_pair(sg, t):
            ng = 1 if 2 * sg + 1 >= G else 2
            return x_sb.ap()[:, sg % XB, 0:ng, t:t + 5:4, :]

        def ohpair(g):
            return oh_sb.ap()[:, g]

        @block.gpsimd
        def _(gpsimd):
            gpsimd.memset(warm_sb.ap(), 0.0).then_inc(sem_warm, 1)

        @block.sync
        def _(sync):
            for sg in range(G2):
                if sg >= XB:
                    done = 2 * (sg - XB) + 2   # groups of buffer sg-XB
                    sync.wait_ge(sem_pe, done)
                    sync.wait_ge(sem_sqd, (sg - XB) + 1)
                    sync.wait_ge(sem_sqa, (sg - XB) + 1)
                if sg == 0:
                    # split: get group 0 to the engines ASAP
                    sync.dma_start(
                        out=x_sb.ap()[:, 0, 0],
                        in_=feat_h.ap()[0][:, 0:LINE]).then_inc(
                        sem_xs[0], 16)
                    sync.dma_start(
                        out=x_sb.ap()[:, 0, 1],
                        in_=feat_h.ap()[0][:, LINE:]).then_inc(sem_g1, 16)
                if sg == 0:
                    sync.dma_start(out=oh_sb.ap(), in_=oh_h.ap()).then_inc(
                        sem_oh, 16)
                    sync.dma_start(out=id_sb.ap(), in_=id_h.ap()).then_inc(
                        sem_id, 16)
                elif 2 * sg + 1 >= G and G % 2 == 1:
                    # last (odd) group: transfer only the real half-line
                    sync.dma_start(
                        out=x_sb.ap()[:, sg % XB, 0],
                        in_=feat_h.ap()[sg][:, 0:LINE]).then_inc(
                        sem_xs[sg % XB], 16)
                else:
                    sync.dma_start(out=x_sb.ap()[:, sg % XB],
                                   in_=feat_h.ap()[sg]).then_inc(
                        sem_xs[sg % XB], 16)
            sync.wait_ge(sem_cp, 1)
            sync.dma_start(out=out_h.ap(), in_=out_sb.ap()).then_inc(
                sem_out, 16)
            sync.wait_ge(sem_out, 16)

        @block.vector
        def _(vector):
            with nc.allow_low_precision("bf16 dump; accums are f32"):
                for sg in range(G2):
                    wait_x(vector, 2 * sg)
                    if 2 * sg + 1 < G:
                        wait_x(vector, 2 * sg + 1)
                    if sg >= 2:
                        vector.wait_ge(sem_sqd, sg - 1)  # dump slot WAW
                    ng = 1 if 2 * sg + 1 >= G else 2
                    xp = xsg_pair(sg, 2)
                    vector.scalar_tensor_tensor(
                        out=dumpd_sb.ap()[:, sg % 2, 0:ng], in0=xp,
                        scalar=1.0,
                        in1=xp, op0=mybir.AluOpType.mult,
                        op1=mybir.AluOpType.mult,
                        accum_out=accd_sb.ap()[:, sg:sg + 1],
                    ).then_inc(sem_sqd, 1)
                # tail. The accum reductions only need DVE/ACT completion,
                # so they overlap the final class matmuls; the psum reads
                # then gate on sem_pe.
                vector.wait_ge(sem_sqa, G2)
                vector.wait_ge(sem_sqd, G2)
                vector.tensor_reduce(
                    out=out_sb.ap()[:, D + 1:D + 2], in_=acca_sb.ap(),
                    axis=mybir.AxisListType.X, op=mybir.AluOpType.add,
                )
                vector.tensor_reduce(
                    out=out_sb.ap()[:, D + 2:D + 3], in_=accd_sb.ap(),
                    axis=mybir.AxisListType.X, op=mybir.AluOpType.add,
                )
                vector.wait_ge(sem_pe, G)
                vector.wait_ge(sem_id, 16)
                vector.tensor_copy(out=gd_sb.ap(),
                                   in_=psum_gram.ap()).then_inc(sem_gd, 1)
                vector.tensor_copy(out=out_sb.ap()[:, 0:HALF],
                                   in_=psum_cls.ap()[:, 0:HALF])
                vector.tensor_copy(out=out_sb.ap()[:, HALF:D],
                                   in_=psum_cls.ap()[:, HALF:D])
                vector.wait_ge(sem_gd, 1)
                vector.scalar_tensor_tensor(
                    out=gd2_sb.ap(), in0=gd_sb.ap(), scalar=1.0,
                    in1=id_sb.ap(), op0=mybir.AluOpType.mult,
                    op1=mybir.AluOpType.mult,
                    accum_out=out_sb.ap()[:, D:D + 1],
                ).then_inc(sem_cp, 1)

        @block.scalar
        def _(scalar):
            with nc.allow_low_precision("bf16 dump; accums are f32"):
                for sg in range(G2):
                    wait_x(scalar, 2 * sg)
                    if 2 * sg + 1 < G:
                        wait_x(scalar, 2 * sg + 1)
                    if sg >= 2:
                        scalar.wait_ge(sem_sqa, sg - 1)  # dump slot WAW
                    ng = 1 if 2 * sg + 1 >= G else 2
                    dout = dumpa_sb.ap()[:, sg % 2, 0:ng * 2 * D].rearrange(
                        "p (a b c) -> p a b c", a=ng, b=2)
                    scalar.activation(
                        dout, xsg_pair(sg, 3),
                        mybir.ActivationFunctionType.Square,
                        accum_out=acca_sb.ap()[:, sg:sg + 1],
                    ).then_inc(sem_sqa, 1)

        @block.tensor
        def _(tensor):
            # clock-ramp warmup: garbage accumulation chain, never read
            tensor.wait_ge(sem_warm, 1)
            for w in range(NWARM):
                tensor.matmul(
                    out=psum_warm.ap()[0:64, :], lhsT=warm_sb.ap()[:, 0:64],
                    rhs=warm_sb.ap(),
                    start=(w == 0), stop=(w == NWARM - 1),
                )
            for g in range(G):
                wait_x(tensor, g)
                # gram pairs 0,1: 4 diag-block DR matmuls each
                for t in range(2):
                    for b in range(4):
                        tensor.matmul(
                            out=psum_gram.ap(),
                            lhsT=xpair_blk(g, t, b),
                            rhs=xpair_blk(g, t, b),
                            start=(g == 0 and t == 0 and b == 0),
                            stop=(g == G - 1 and t == 1 and b == 3),
                            perf_mode=DR,
                        )
                # class sums: 4 DR matmuls sharing the shipped one-hot
                if g == 0:
                    tensor.wait_ge(sem_oh, 16)
                last = None
                for t in range(4):
                    last = tensor.matmul(
                        out=psum_cls.ap(), lhsT=ohpair(g),
                        rhs=xpair(g, t),
                        start=(g == 0 and t == 0),
                        stop=(g == G - 1 and t == 3),
                        perf_mode=DR,
                    )
                last.then_inc(sem_pe, 1)

    nc.compile()
    return nc


def _finalize(results, labels: np.ndarray, C: int, N: int):
    sums = np.concatenate(
        [np.asarray(r["out"][:, :D], dtype=np.float64) for r in results], axis=0
    )  # [1024, D]
    ssq = float(sum(np.asarray(r["out"][:, D:], dtype=np.float64).sum()
                    for r in results))
    counts = np.bincount(labels, minlength=CORES * P).astype(np.float64)

    sums = sums[:C]
    counts = counts[:C]
    means = sums / counts[:, None]
    g = sums.sum(axis=0) / N
    tr_sw = ssq - float(((sums * sums).sum(axis=1) / counts).sum())
    tr_sb = float(((means - g) ** 2).sum())
    return np.asarray(np.float32(tr_sw / tr_sb))


def run(features, labels, num_classes, trace=False):
    features = np.asarray(features, dtype=np.float32)
    labels = np.asarray(labels).astype(np.int64).ravel()
    C = int(num_classes)
    N = features.shape[0]
    assert C <= CORES * P, f"num_classes={C} exceeds {CORES * P}"

    if trace:
        _ensure_ntff_hook()
    in_maps, G = _host_shard(features, labels)
    nc = _build_raw(G)
    res = run_bass_kernel_spmd(nc, in_maps, list(range(CORES)), trace=trace)
    out = _finalize(res.results, labels, C, N)
    return out, res


def kernel(**inputs) -> np.ndarray:
    trace = os.environ.get("KERNEL_TRACE", "0") == "1"
    out, _ = run(inputs["features"], inputs["labels"], inputs["num_classes"],
                 trace=trace)
    return out
